# revision 30
# baseline (speedup 1.0000x reference)
"""BertImageSelfAttention Trainium2 kernel (v2: quadrant-packed attention).

Shapes (fixed): hidden_states [4, 2048, 1024], 16 heads x 64, text [4, 64, 768].
Sharding: 8 cores = 4 batches x 2 head-groups (8 heads each). Each core computes
its batch's attention context for its 8 heads; host reassembles [4, 2048, 1024].

v2 design (measured on HW: bf16 matmul 224ns/512-col; two K=64 matmuls on
disjoint PE row-quadrants run CONCURRENTLY at 118ns each; DVE tensor_scalar
fp32->int16 is round-to-nearest):

  A. Head-PAIR attention: heads 2hp/2hp+1 live on partition halves of the
     QT/KTp hp-chunks (no zero padding). Score matmuls for the pair are
     row-quadrant packed -> 2x score throughput (115us -> 59us of PE).
  B. Exp split across engines: per (head, t-chunk) tile [128,1024],
     t in {2,5,8,11,14} -> DVE Schraudolph (int16 bits of bf16 exp:
     bits = score*16*log2e + [128*(127 + amask*log2e) - 5.5], rint, clamp>=0
     implicit via sign bit -> negative ~ -0; amask==0 graded), rest -> ACT
     Exp (bias=amask, scale=0.125). ~176 ACT / ~80 DVE tiles -> neither
     engine binds (PE does, at ~270us).
  C. et tiles persist in SBUF [128, 2, 16, 1024] bf16 per block; ctx for
     head A accumulates in ctxp PSUM dosed at iters 9..16 of its own block,
     head B's ctx doses into the NEXT block's iters 1..8 (block 7's B uses
     the then-idle pjp pool). PSUM: scp [128,2,1024] single-buffered pair
     (4 banks) + ctxp 2 + pjp 2 = 8.
  D. Normalize pairs {A(n), B(n-1)}: one [128,16] denom gather (2 DMAs),
     bf16 reciprocal, scatter to rows 0/64 of rrow, quadrant-packed PE
     broadcast outer products into freed ctxp slots, multiply+bias on
     GpSimd, out DMA [64,512] fp32 on the SWDGE queue. cs evictions bf16.
  E. Loads spread over FOUR DMA queues in first-need order (wv no longer
     gates the first ctx by a whole queue depth).
"""

import os
from collections import defaultdict

import numpy as np
import ml_dtypes

import concourse.bass as bass
import concourse.bacc as bacc
import concourse.tile as tile
from concourse import mybir
from concourse.bass_utils import run_bass_kernel_spmd

P = 128
B, S, DV = 4, 2048, 1024
H, Dh = 16, 64
T, DT = 64, 768
NCORES = 8
E = 512          # head-group width (8 heads x 64)
CC = DV // P     # 8 contraction chunks for projections
ECH = E // P     # 4 e-chunks == head pairs
HP = 4           # head pairs per core
DC = DT // P     # 6 text-dim chunks
SC = S // P      # 16 seq chunks of 128
SBL = S // 512   # 4 seq blocks of 512
HPC = 8          # heads per core

FP32 = mybir.dt.float32
BF16 = mybir.dt.bfloat16
I16 = mybir.dt.int16
AF = mybir.ActivationFunctionType
OP = mybir.AluOpType

BF16_NP = ml_dtypes.bfloat16

# Schraudolph constants: bf16 bits of exp(score*0.125 + amask)
SCHR_S1 = 16.0 * 1.4426950408889634          # 0.125 * 128 * log2(e)
SCHR_B0 = 128.0 * 127.0 - 5.5                # bias bits + centering
SCHR_AM = 128.0 * 1.4426950408889634         # amask coefficient
# exp split: head A (par 0) -> ACT exact exp; head B (par 1) -> DVE
# Schraudolph, except ACT_B_T chunks which ACT absorbs for load balance.
ACT_B_T = (0, 4, 9, 13)

_CACHE = {}

last_results = None


def _emit(tc, aps):
    nc = tc.nc
    xT = aps["xT"].rearrange("(c p) s -> p c s", p=P)          # [128, 8, 2048]
    wq = aps["wq"].rearrange("(c p) e -> p c e", p=P)          # [128, 8, 512]
    wk = aps["wk"].rearrange("(c p) e -> p c e", p=P)
    wv = aps["wv"].rearrange("(c p) e -> p c e", p=P)
    wdq = aps["wdq"].rearrange("(c p) e -> p c e", p=P)        # [128, 6, 512]
    wdk = aps["wdk"].rearrange("(c p) e -> p c e", p=P)
    txt = aps["txt"]                                           # [64, 768] bf16
    smallpack = aps["smallpack"]                               # [128, 52]
    out = aps["out"]                                           # [8, 64, 2048] f32

    from contextlib import ExitStack

    with ExitStack() as ctx:
        wpool = ctx.enter_context(tc.tile_pool(name="wpool", bufs=1))
        xpool = ctx.enter_context(tc.tile_pool(name="xpool", bufs=1))
        qkpool = ctx.enter_context(tc.tile_pool(name="qkpool", bufs=1))
        vpool = ctx.enter_context(tc.tile_pool(name="vpool", bufs=1))
        etp = ctx.enter_context(tc.tile_pool(name="etp", bufs=1))
        rbp = ctx.enter_context(tc.tile_pool(name="rbp", bufs=4))
        outp = ctx.enter_context(tc.tile_pool(name="outp", bufs=4))
        smallp = ctx.enter_context(tc.tile_pool(name="smallp", bufs=1))
        rcp = ctx.enter_context(tc.tile_pool(name="rcp", bufs=2))
        # PSUM: 8 banks = scp 2 x [128, 2, 512] (one j-half of the A/B score
        # pair; 2-deep so exp(t,j) overlaps scores(t,j+1))
        # + ctxp 2 x 1 bank (ctx accumulators / phase-E broadcasts)
        # + pjp 2 x 1 bank (projection scratch; block-7 ctx-B).
        ctxp = ctx.enter_context(tc.tile_pool(name="ctxp", bufs=2, space="PSUM"))
        pjp = ctx.enter_context(tc.tile_pool(name="pjp", bufs=2, space="PSUM"))
        scp = ctx.enter_context(tc.tile_pool(name="scp", bufs=2, space="PSUM"))

        # ---- gate-critical small loads (scalar queue, nothing big ahead) ----
        txt_sb = smallp.tile([P, DT], BF16, tag="txt")
        nc.vector.memset(txt_sb[T:P, :], 0.0)
        nc.scalar.dma_start(out=txt_sb[0:T, :], in_=txt)
        spk = smallp.tile([P, 2 * SC + 5 * ECH + 1], FP32, tag="spk")
        nc.scalar.dma_start(out=spk, in_=smallpack)
        # tmask rides in smallpack (a 128-byte DMA can strand for 20us behind
        # a big transfer on the same HWDGE queue); cast it into the zeroed
        # [128,128] stationary here
        tmask_sb = smallp.tile([P, P], BF16, tag="tmask")
        nc.vector.memset(tmask_sb, 0.0)
        nc.vector.tensor_copy(tmask_sb[0:T, 0:1],
                              spk[0:T, 2 * SC + 5 * ECH:2 * SC + 5 * ECH + 1])
        ones_sb = smallp.tile([P, 1], BF16, tag="ones")
        nc.vector.memset(ones_sb, 1.0)
        ones2 = smallp.tile([P, Dh], BF16, tag="ones2")
        nc.vector.memset(ones2, 1.0)
        amask_sb = spk[:, 0:SC]
        bq_sb = spk[:, SC:SC + ECH]
        bk_sb = spk[:, SC + ECH:SC + 2 * ECH]
        bdq_sb = spk[:, SC + 2 * ECH:SC + 3 * ECH]
        bdk_sb = spk[:, SC + 3 * ECH:SC + 4 * ECH]
        bvT_sb = spk[:, SC + 4 * ECH:SC + 5 * ECH]
        s2_sb = spk[:, SC + 5 * ECH:2 * SC + 5 * ECH]   # Schraudolph bias/chunk

        # ---- big loads, four DMA queues, first-need order ----
        xT_sb = xpool.tile([P, CC, S], BF16, tag="xT")
        wq_sb = wpool.tile([P, CC, E], BF16, tag="wq")
        wk_sb = wpool.tile([P, CC, E], BF16, tag="wk")
        wv_sb = wpool.tile([P, CC, E], BF16, tag="wv")
        wdq_sb = wpool.tile([P, DC, E], BF16, tag="wdq")
        wdk_sb = wpool.tile([P, DC, E], BF16, tag="wdk")
        # three queues (~90 GB/s each measured), first-need order:
        #   scalar: txt, spk, xT0 upper half, wq-e01, xT-ss3
        #   sync:   wdq, wdk (gate critical path), wk-e01, wv, e23 halves
        #   gpsimd: xT0 lower half, xT-ss1, xT-ss2
        eh0 = slice(0, 256)
        eh1 = slice(256, 512)
        ch0 = slice(0, 4)
        ch1 = slice(4, 8)
        sl0 = slice(0, 512)
        sl1 = slice(512, 1024)
        sl2 = slice(1024, 1536)
        sl3 = slice(1536, 2048)
        nc.sync.dma_start(out=wdq_sb, in_=wdq)
        nc.sync.dma_start(out=wdk_sb, in_=wdk)
        nc.scalar.dma_start(out=xT_sb[:, ch1, sl0], in_=xT[:, ch1, sl0])
        nc.gpsimd.dma_start(out=xT_sb[:, ch0, sl0], in_=xT[:, ch0, sl0])
        nc.scalar.dma_start(out=wq_sb[:, :, eh0], in_=wq[:, :, eh0])
        nc.sync.dma_start(out=wk_sb[:, :, eh0], in_=wk[:, :, eh0])
        nc.gpsimd.dma_start(out=xT_sb[:, :, sl1], in_=xT[:, :, sl1])
        nc.scalar.dma_start(out=xT_sb[:, :, sl3], in_=xT[:, :, sl3])
        nc.gpsimd.dma_start(out=xT_sb[:, :, sl2], in_=xT[:, :, sl2])
        nc.sync.dma_start(out=wv_sb, in_=wv)
        nc.sync.dma_start(out=wq_sb[:, :, eh1], in_=wq[:, :, eh1])
        nc.sync.dma_start(out=wk_sb[:, :, eh1], in_=wk[:, :, eh1])

        # ---- phase A: pooled text + gates (as v1) ----
        pps = pjp.tile([P, 512], FP32, tag="acc", name="poolps")
        for c in range(DC):
            nc.tensor.matmul(pps[:, c:c + 1],
                             lhsT=txt_sb[:, c * P:(c + 1) * P],
                             rhs=tmask_sb[:, 0:1], start=True, stop=True)
        nc.tensor.matmul(pps[:, DC:DC + 1], lhsT=tmask_sb, rhs=ones_sb,
                         start=True, stop=True)
        # partition-broadcast 1/sum(mask) via a tiny PE outer product — a DMA
        # here would strand behind the big preloads on every queue
        rmsum = smallp.tile([1, 1], BF16, tag="rmsum")
        with nc.allow_low_precision(reason="bf16 mask-count recip"):
            nc.vector.reciprocal(rmsum, pps[0:1, DC:DC + 1])
        ones_row = smallp.tile([1, P], BF16, tag="ones_row")
        nc.vector.memset(ones_row, 1.0)
        rmb_ps = pjp.tile([P, 512], FP32, tag="acc", name="rmbps")
        nc.tensor.matmul(rmb_ps[:, 0:1], lhsT=ones_row, rhs=rmsum,
                         start=True, stop=True)
        rmsumb = smallp.tile([P, 1], FP32, tag="rmsumb")
        nc.vector.tensor_copy(rmsumb, rmb_ps[:, 0:1])
        poolT = smallp.tile([P, DC], BF16, tag="poolT")
        nc.vector.tensor_copy(poolT, pps[:, 0:DC])

        gq_sb = smallp.tile([P, ECH], FP32, tag="gq")
        gk_sb = smallp.tile([P, ECH], FP32, tag="gk")
        for (nm, wd_sb, bd_sb, g_sb) in (
            ("q", wdq_sb, bdq_sb, gq_sb),
            ("k", wdk_sb, bdk_sb, gk_sb),
        ):
            for ec in range(ECH):
                gp = pjp.tile([P, 512], FP32, tag="acc", name=f"g{nm}{ec}")
                for c in range(DC):
                    nc.tensor.matmul(
                        gp[:, 0:1],
                        lhsT=wd_sb[:, c, ec * P:(ec + 1) * P],
                        rhs=poolT[:, c:c + 1],
                        start=(c == 0), stop=(c == DC - 1),
                    )
                nc.scalar.activation(g_sb[:, ec:ec + 1], gp[:, 0:1], AF.Sigmoid,
                                     bias=bd_sb[:, ec:ec + 1], scale=rmsumb)
            nc.vector.tensor_scalar(g_sb, g_sb, 1.0, None, OP.add)
        # fold BOTH gates into Q: scores = (xWk+bk) . [gqk*(xWq+bq)]
        gqk_sb = smallp.tile([P, ECH], FP32, tag="gqk")
        nc.vector.tensor_mul(gqk_sb, gq_sb, gk_sb)
        gqkbq_sb = smallp.tile([P, ECH], FP32, tag="gqkbq")
        nc.vector.tensor_mul(gqkbq_sb, gqk_sb, bq_sb)

        # ---- projection targets + emitters ----
        QT = qkpool.tile([P, HP, S], BF16, tag="QT")
        KTp = qkpool.tile([P, HP, S], BF16, tag="KTp")   # pair halves, no pad
        Vaug = vpool.tile([P, SC, HPC, Dh + 1], BF16, tag="Vaug")
        etAB = etp.tile([P, 2, SC, 1024], BF16, tag="etAB")

        def emit_qk_half(ec, ss, which):
            sl = slice(ss * 512, (ss + 1) * 512)
            if which == "q":
                ps = pjp.tile([P, 512], FP32, tag="acc", name=f"psq{ec}_{ss}")
                for c in range(CC):
                    nc.tensor.matmul(
                        ps,
                        lhsT=wq_sb[:, c, ec * P:(ec + 1) * P],
                        rhs=xT_sb[:, c, sl],
                        start=(c == 0), stop=(c == CC - 1),
                    )
                nc.scalar.activation(
                    QT[:, ec, sl], ps, AF.Identity,
                    bias=gqkbq_sb[:, ec:ec + 1], scale=gqk_sb[:, ec:ec + 1],
                )
            else:
                psk = pjp.tile([P, 512], FP32, tag="acc", name=f"psk{ec}_{ss}")
                for c in range(CC):
                    nc.tensor.matmul(
                        psk,
                        lhsT=wk_sb[:, c, ec * P:(ec + 1) * P],
                        rhs=xT_sb[:, c, sl],
                        start=(c == 0), stop=(c == CC - 1),
                    )
                nc.vector.tensor_scalar(
                    KTp[:, ec, sl], psk, bk_sb[:, ec:ec + 1], None, OP.add)

        def emit_v(t):
            ps = pjp.tile([P, 512], FP32, tag="acc", name=f"psv{t}")
            for c in range(CC):
                nc.tensor.matmul(
                    ps,
                    lhsT=xT_sb[:, c, t * P:(t + 1) * P],
                    rhs=wv_sb[:, c, :],
                    start=(c == 0), stop=(c == CC - 1),
                )
            nc.vector.tensor_copy(
                Vaug[:, t, :, 0:Dh],
                ps.rearrange("p (h d) -> p h d", h=HPC),
            )
            nc.vector.memset(Vaug[:, t, :, Dh:Dh + 1], 1.0)

        # only the first block's score inputs run up front (K before Q:
        # K eviction is gate-free so it can start while gates resolve)
        emit_qk_half(0, 0, "k")
        for ec, ss, which in ((0, 0, "q"), (0, 1, "q")):
            emit_qk_half(ec, ss, which)

        # ---- projection dose map: (block, iter) -> emitters ----
        pdose = defaultdict(list)

        def addp(n, i, fn):
            pdose[(n, min(i, 16))].append(fn)

        # block 0 consumes seq-blocks in DMA arrival order: ss0, ss3, ss1, ss2
        PERM0 = (0, 1, 2, 3, 12, 13, 14, 15, 4, 5, 6, 7, 8, 9, 10, 11)
        addp(0, 3, lambda: emit_qk_half(0, 3, "k"))
        addp(0, 5, lambda: emit_qk_half(0, 2, "q"))
        addp(0, 7, lambda: emit_qk_half(0, 1, "k"))
        addp(0, 8, lambda: emit_qk_half(0, 3, "q"))
        addp(0, 11, lambda: emit_qk_half(0, 2, "k"))
        for i in range(SC):
            addp(0, 6 + (i * 9) // 16, lambda t=PERM0[i]: emit_v(t))
        for g in (1, 2, 3):
            bb = 2 * g - 1
            for k in range(4):
                addp(bb, 2 + 2 * k, lambda g=g, k=k: emit_qk_half(g, k, "k"))
            addp(bb, 12, lambda g=g: emit_qk_half(g, 0, "q"))
            addp(bb, 13 if g == 3 else 14, lambda g=g: emit_qk_half(g, 1, "q"))
            if g < 3:
                addp(2 * g, 9, lambda g=g: emit_qk_half(g, 2, "q"))
                addp(2 * g, 11, lambda g=g: emit_qk_half(g, 3, "q"))
            else:
                addp(5, 14, lambda: emit_qk_half(3, 2, "q"))
                addp(5, 15, lambda: emit_qk_half(3, 3, "q"))

        # ---- ctx dose iteration maps (by emit INDEX into the block's perm) --
        def a_iter(i):
            return max(9 + (i * 7) // 16, i + 1)

        def b_iter(i):
            # MUST be <= position of the overwriting exp: block n's ctx-B
            # unit i (t = perm[i]) is emitted in block n+1 before
            # exp-B(n+1, t) overwrites etAB[:,1,t] (program-order WAR).
            return i // 2

        def perm_of(n):
            return PERM0 if n == 0 else tuple(range(SC))

        # ---- attention ----
        ctx_ab = {}      # (n, 'A'/'B') -> [j0_tile, j1_tile]
        cs_store = {}    # (n, 'A'/'B') -> (head, sp, cs_tile)

        def ctx_unit(n, ab, i, pool):
            hp, sp = n // 2, n % 2
            h = 2 * hp + (0 if ab == "A" else 1)
            t = perm_of(n)[i]
            key = (n, ab)
            if key not in ctx_ab:
                ctx_ab[key] = [
                    pool.tile([P, 512], FP32, tag="acc",
                              name=f"ctx{n}{ab}{j}")
                    for j in range(2)]
            for j in range(2):
                nc.tensor.matmul(
                    ctx_ab[key][j][0:Dh + 1, :],
                    lhsT=Vaug[:, t, h, :],
                    rhs=etAB[:, h % 2, t, j * 512:(j + 1) * 512],
                    start=(i == 0), stop=(i == SC - 1),
                )

        def ctx_evict(n, ab):
            hp, sp = n // 2, n % 2
            h = 2 * hp + (0 if ab == "A" else 1)
            cs = rbp.tile([Dh + 1, 1024], BF16, tag="cs")
            for j in range(2):
                nc.vector.tensor_copy(
                    cs[:, j * 512:(j + 1) * 512], ctx_ab[(n, ab)][j][0:Dh + 1, :])
            cs_store[(n, ab)] = (h, sp, cs)

        def phase_e(units):
            """units: list of (h, sp, cs, rowbase in {0,64}).

            The denominator row (cs row 64, bf16 SBUF) is PE-broadcast
            directly into a PSUM tile (64 rows per unit), reciprocal'd in
            fp32 on DVE, then ctx*recip on DVE + bias on ACT. No DMAs.
            """
            for j in range(2):
                rcb = ctxp.tile([P, 512], FP32, tag="acc",
                                name=f"rcb{units[0][0]}_{units[0][1]}_{j}")
                for (h, sp, cs, rb) in units:
                    nc.tensor.matmul(
                        rcb[rb:rb + Dh, :], lhsT=ones2[Dh:Dh + 1, :],
                        rhs=cs[Dh:Dh + 1, j * 512:(j + 1) * 512],
                        start=True, stop=True,
                        tile_position=(Dh, rb),
                    )
                for (h, sp, cs, rb) in units:
                    hp2, hi = h // 2, h % 2
                    # per-unit reciprocal lands at partition base 0 so the
                    # multiply's SBUF operands share a base partition
                    rci = rcp.tile([Dh, 512], FP32, tag="rci")
                    nc.vector.reciprocal(rci, rcb[rb:rb + Dh, :])
                    ot = outp.tile([Dh, 512], FP32, tag="outsb")
                    nc.vector.tensor_mul(
                        ot, cs[0:Dh, j * 512:(j + 1) * 512], rci)
                    nc.scalar.activation(
                        ot, ot, AF.Identity,
                        bias=bvT_sb[hi * Dh:(hi + 1) * Dh, hp2:hp2 + 1],
                    )
                    nc.gpsimd.dma_start(
                        out=out[h, :, sp * 1024 + j * 512:
                                sp * 1024 + (j + 1) * 512], in_=ot)

        for n in range(8):
            hp, sp = n // 2, n % 2
            perm = perm_of(n)
            for pos in range(SC):
                t = perm[pos]
                with tc.high_priority(offset=-100):
                    if pos == 0 and n >= 1:
                        ctx_evict(n - 1, "A")
                    if n >= 1:
                        for ib in range(SC):
                            if b_iter(ib) == pos:
                                ctx_unit(n - 1, "B", ib, ctxp)
                    if n == 7 and 1 <= pos <= 15:
                        ctx_unit(7, "B", pos - 1, pjp)
                    if pos == 9 and n >= 1:
                        # normalize the whole previous block: {A(n-1), B(n-1)}
                        ctx_evict(n - 1, "B")
                        hA, spA, csA = cs_store.pop((n - 1, "A"))
                        hB, spB, csB = cs_store.pop((n - 1, "B"))
                        phase_e([(hA, spA, csA, 0), (hB, spB, csB, 64)])
                    for fn in pdose.get((n, pos), ()):
                        fn()
                    for ia in range(SC):
                        if a_iter(ia) == pos:
                            ctx_unit(n, "A", ia, ctxp)
                # scores: packed pairs (parity = PE row quadrant), j-split so
                # exp(t,j) pipelines against scores of the next j-half
                for j in range(2):
                    sps = scp.tile([P, 2, 512], FP32, tag="sc")
                    for par in range(2):
                        pp = slice(par * 64, (par + 1) * 64)
                        s0 = sp * 1024 + j * 512
                        # explicit quadrant: pool-tile reg offsets would
                        # otherwise force tile_position (0,0) = no packing
                        nc.tensor.matmul(
                            sps[:, par, :],
                            lhsT=KTp[pp, hp, t * P:(t + 1) * P],
                            rhs=QT[pp, hp, s0:s0 + 512],
                            start=True, stop=True,
                            tile_position=(par * 64, 0),
                        )
                    ets = slice(j * 512, (j + 1) * 512)
                    for par in range(2):
                        if par == 1 and t not in ACT_B_T:
                            nc.vector.tensor_scalar(
                                etAB[:, par, t, ets].bitcast(I16),
                                sps[:, par, :],
                                SCHR_S1, s2_sb[:, t:t + 1], OP.mult, OP.add)
                        else:
                            nc.scalar.activation(
                                etAB[:, par, t, ets], sps[:, par, :], AF.Exp,
                                bias=amask_sb[:, t:t + 1], scale=0.125)
            # end of t loop: position-16 doses
            with tc.high_priority(offset=-100):
                for ia in range(SC):
                    if a_iter(ia) == 16:
                        ctx_unit(n, "A", ia, ctxp)
                for fn in pdose.get((n, 16), ()):
                    fn()
                if n == 7:
                    ctx_unit(7, "B", 15, pjp)

        # tail: last evictions + one paired phase-E
        ctx_evict(7, "A")
        ctx_evict(7, "B")
        hA, spA, csA = cs_store.pop((7, "A"))
        hB, spB, csB = cs_store.pop((7, "B"))
        phase_e([(hA, spA, csA, 0), (hB, spB, csB, 64)])


def _build():
    key = "nc"
    if key in _CACHE:
        return _CACHE[key]
    nc = bacc.Bacc("TRN2", target_bir_lowering=False, debug=False,
                   enable_asserts=False)
    aps = {}

    def din(name, shape, dt):
        aps[name] = nc.dram_tensor(name, shape, dt, kind="ExternalInput").ap()

    din("xT", [DV, S], BF16)
    din("wq", [DV, E], BF16)
    din("wk", [DV, E], BF16)
    din("wv", [DV, E], BF16)
    din("wdq", [DT, E], BF16)
    din("wdk", [DT, E], BF16)
    din("txt", [T, DT], BF16)
    # amask[0:16] | bq | bk | bdq | bdk | bv | schraudolph-s2[36:52] | tmask
    din("smallpack", [P, 2 * SC + 5 * ECH + 1], FP32)
    aps["out"] = nc.dram_tensor("out", [HPC, Dh, S], FP32,
                                kind="ExternalOutput").ap()

    with tile.TileContext(nc) as tc:
        _emit(tc, aps)
    nc.compile()
    _CACHE[key] = nc
    return nc


def kernel(**inputs):
    global last_results
    hs = np.asarray(inputs["hidden_states"], dtype=np.float32)
    amask = np.asarray(inputs["attention_mask"], dtype=np.float32)
    txt = np.asarray(inputs["txt_embedding"], dtype=np.float32)
    tmask = np.asarray(inputs["txt_attention_mask"], dtype=np.float32)
    Wq = np.asarray(inputs["Wq"], dtype=np.float32)
    Wk = np.asarray(inputs["Wk"], dtype=np.float32)
    Wv = np.asarray(inputs["Wv"], dtype=np.float32)
    Wdq = np.asarray(inputs["Wdq"], dtype=np.float32)
    Wdk = np.asarray(inputs["Wdk"], dtype=np.float32)
    bq = np.asarray(inputs["bq"], dtype=np.float32)
    bk = np.asarray(inputs["bk"], dtype=np.float32)
    bv = np.asarray(inputs["bv"], dtype=np.float32)
    bdq = np.asarray(inputs["bdq"], dtype=np.float32)
    bdk = np.asarray(inputs["bdk"], dtype=np.float32)

    nc = _build()

    in_maps = []
    for c in range(NCORES):
        b, g = c // 2, c % 2
        cols = slice(g * E, (g + 1) * E)
        amask_t = amask[b, 0, 0].reshape(SC, P).T      # [128, 16]
        in_maps.append({
            "xT": np.ascontiguousarray(hs[b].T).astype(BF16_NP),
            "wq": Wq[:, cols].astype(BF16_NP),
            "wk": Wk[:, cols].astype(BF16_NP),
            "wv": Wv[:, cols].astype(BF16_NP),
            "wdq": Wdq[:, cols].astype(BF16_NP),
            "wdk": Wdk[:, cols].astype(BF16_NP),
            "txt": txt[b].astype(BF16_NP),
            "smallpack": np.ascontiguousarray(np.concatenate([
                amask_t,
                bq[cols].reshape(ECH, P).T,
                bk[cols].reshape(ECH, P).T,
                bdq[cols].reshape(ECH, P).T,
                bdk[cols].reshape(ECH, P).T,
                bv[cols].reshape(ECH, P).T,
                amask_t * SCHR_AM + SCHR_B0,
                np.pad(tmask[b, :, 0], (0, P - T)).reshape(P, 1),
            ], axis=1).astype(np.float32)),
        })

    tr = int(os.environ.get("BASS_KERNEL_TRACE", "0"))
    if tr == 2:
        run_bass_kernel_spmd(nc, in_maps, list(range(NCORES)), trace=False)
    res = run_bass_kernel_spmd(nc, in_maps, list(range(NCORES)), trace=bool(tr))
    last_results = res

    outp = np.empty((B, S, DV), dtype=np.float32)
    for c in range(NCORES):
        b, g = c // 2, c % 2
        co = res.results[c]["out"].transpose(2, 0, 1).reshape(S, E)
        outp[b, :, g * E:(g + 1) * E] = co
    return outp


# revision 31
# speedup vs baseline: 1.2127x; 1.2127x over previous
"""BertImageSelfAttention Trainium2 kernel (v2: quadrant-packed attention).

Shapes (fixed): hidden_states [4, 2048, 1024], 16 heads x 64, text [4, 64, 768].
Sharding: 8 cores = 4 batches x 2 head-groups (8 heads each). Each core computes
its batch's attention context for its 8 heads; host reassembles [4, 2048, 1024].

v2 design (measured on HW: bf16 matmul 224ns/512-col; two K=64 matmuls on
disjoint PE row-quadrants run CONCURRENTLY at 118ns each; DVE tensor_scalar
fp32->int16 is round-to-nearest):

  A. Head-PAIR attention: heads 2hp/2hp+1 live on partition halves of the
     QT/KTp hp-chunks (no zero padding). Score matmuls for the pair are
     row-quadrant packed -> 2x score throughput (115us -> 59us of PE).
  B. Exp split across engines: per (head, t-chunk) tile [128,1024],
     t in {2,5,8,11,14} -> DVE Schraudolph (int16 bits of bf16 exp:
     bits = score*16*log2e + [128*(127 + amask*log2e) - 5.5], rint, clamp>=0
     implicit via sign bit -> negative ~ -0; amask==0 graded), rest -> ACT
     Exp (bias=amask, scale=0.125). ~176 ACT / ~80 DVE tiles -> neither
     engine binds (PE does, at ~270us).
  C. et tiles persist in SBUF [128, 2, 16, 1024] bf16 per block; ctx for
     head A accumulates in ctxp PSUM dosed at iters 9..16 of its own block,
     head B's ctx doses into the NEXT block's iters 1..8 (block 7's B uses
     the then-idle pjp pool). PSUM: scp [128,2,1024] single-buffered pair
     (4 banks) + ctxp 2 + pjp 2 = 8.
  D. Normalize pairs {A(n), B(n-1)}: one [128,16] denom gather (2 DMAs),
     bf16 reciprocal, scatter to rows 0/64 of rrow, quadrant-packed PE
     broadcast outer products into freed ctxp slots, multiply+bias on
     GpSimd, out DMA [64,512] fp32 on the SWDGE queue. cs evictions bf16.
  E. Loads spread over FOUR DMA queues in first-need order (wv no longer
     gates the first ctx by a whole queue depth).
"""

import os
from collections import defaultdict

import numpy as np
import ml_dtypes

import concourse.bass as bass
import concourse.bacc as bacc
import concourse.tile as tile
from concourse import mybir
from concourse.bass_utils import run_bass_kernel_spmd

P = 128
B, S, DV = 4, 2048, 1024
H, Dh = 16, 64
T, DT = 64, 768
NCORES = 8
E = 512          # head-group width (8 heads x 64)
CC = DV // P     # 8 contraction chunks for projections
ECH = E // P     # 4 e-chunks == head pairs
HP = 4           # head pairs per core
DC = DT // P     # 6 text-dim chunks
SC = S // P      # 16 seq chunks of 128
SBL = S // 512   # 4 seq blocks of 512
HPC = 8          # heads per core

FP32 = mybir.dt.float32
BF16 = mybir.dt.bfloat16
I16 = mybir.dt.int16
AF = mybir.ActivationFunctionType
OP = mybir.AluOpType

BF16_NP = ml_dtypes.bfloat16

# Schraudolph constants: bf16 bits of exp(score*0.125 + amask)
SCHR_S1 = 16.0 * 1.4426950408889634          # 0.125 * 128 * log2(e)
SCHR_B0 = 128.0 * 127.0 - 5.5                # bias bits + centering
SCHR_AM = 128.0 * 1.4426950408889634         # amask coefficient
# exp split: head A (par 0) -> ACT exact exp; head B (par 1) -> DVE
# Schraudolph, except ACT_B_T chunks which ACT absorbs for load balance.
ACT_B_T = (0, 4, 9, 13)

_CACHE = {}

last_results = None


def _emit(tc, aps):
    nc = tc.nc
    xT = aps["xT"].rearrange("(c p) s -> p c s", p=P)          # [128, 8, 2048]
    wq = aps["wq"].rearrange("(c p) e -> p c e", p=P)          # [128, 8, 512]
    wk = aps["wk"].rearrange("(c p) e -> p c e", p=P)
    wv = aps["wv"].rearrange("(c p) e -> p c e", p=P)
    wdq = aps["wdq"].rearrange("(c p) e -> p c e", p=P)        # [128, 6, 512]
    wdk = aps["wdk"].rearrange("(c p) e -> p c e", p=P)
    txt = aps["txt"]                                           # [64, 768] bf16
    smallpack = aps["smallpack"]                               # [128, 52]
    out = aps["out"]                                           # [8, 64, 2048] f32

    from contextlib import ExitStack

    with ExitStack() as ctx:
        wpool = ctx.enter_context(tc.tile_pool(name="wpool", bufs=1))
        xpool = ctx.enter_context(tc.tile_pool(name="xpool", bufs=1))
        qkpool = ctx.enter_context(tc.tile_pool(name="qkpool", bufs=1))
        vpool = ctx.enter_context(tc.tile_pool(name="vpool", bufs=1))
        etp = ctx.enter_context(tc.tile_pool(name="etp", bufs=1))
        rbp = ctx.enter_context(tc.tile_pool(name="rbp", bufs=4))
        outp = ctx.enter_context(tc.tile_pool(name="outp", bufs=4))
        smallp = ctx.enter_context(tc.tile_pool(name="smallp", bufs=1))
        rcp = ctx.enter_context(tc.tile_pool(name="rcp", bufs=2))
        # PSUM: 8 banks = scp 2 x [128, 2, 512] (one j-half of the A/B score
        # pair; 2-deep so exp(t,j) overlaps scores(t,j+1))
        # + ctxp 2 x 1 bank (ctx accumulators / phase-E broadcasts)
        # + pjp 2 x 1 bank (projection scratch; block-7 ctx-B).
        ctxp = ctx.enter_context(tc.tile_pool(name="ctxp", bufs=2, space="PSUM"))
        pjp = ctx.enter_context(tc.tile_pool(name="pjp", bufs=2, space="PSUM"))
        scp = ctx.enter_context(tc.tile_pool(name="scp", bufs=2, space="PSUM"))

        # ---- gate-critical small loads (scalar queue, nothing big ahead) ----
        txt_sb = smallp.tile([P, DT], BF16, tag="txt")
        nc.vector.memset(txt_sb[T:P, :], 0.0)
        nc.scalar.dma_start(out=txt_sb[0:T, :], in_=txt)
        spk = smallp.tile([P, 2 * SC + 5 * ECH + 1], FP32, tag="spk")
        nc.scalar.dma_start(out=spk, in_=smallpack)
        # tmask rides in smallpack (a 128-byte DMA can strand for 20us behind
        # a big transfer on the same HWDGE queue); cast it into the zeroed
        # [128,128] stationary here
        tmask_sb = smallp.tile([P, P], BF16, tag="tmask")
        nc.vector.memset(tmask_sb, 0.0)
        nc.vector.tensor_copy(tmask_sb[0:T, 0:1],
                              spk[0:T, 2 * SC + 5 * ECH:2 * SC + 5 * ECH + 1])
        ones_sb = smallp.tile([P, 1], BF16, tag="ones")
        nc.vector.memset(ones_sb, 1.0)
        ones2 = smallp.tile([P, Dh], BF16, tag="ones2")
        nc.vector.memset(ones2, 1.0)
        amask_sb = spk[:, 0:SC]
        bq_sb = spk[:, SC:SC + ECH]
        bk_sb = spk[:, SC + ECH:SC + 2 * ECH]
        bdq_sb = spk[:, SC + 2 * ECH:SC + 3 * ECH]
        bdk_sb = spk[:, SC + 3 * ECH:SC + 4 * ECH]
        bvT_sb = spk[:, SC + 4 * ECH:SC + 5 * ECH]
        s2_sb = spk[:, SC + 5 * ECH:2 * SC + 5 * ECH]   # Schraudolph bias/chunk

        # ---- big loads, four DMA queues, first-need order ----
        xT_sb = xpool.tile([P, CC, S], BF16, tag="xT")
        wq_sb = wpool.tile([P, CC, E], BF16, tag="wq")
        wk_sb = wpool.tile([P, CC, E], BF16, tag="wk")
        wv_sb = wpool.tile([P, CC, E], BF16, tag="wv")
        wdq_sb = wpool.tile([P, DC, E], BF16, tag="wdq")
        wdk_sb = wpool.tile([P, DC, E], BF16, tag="wdk")
        # three queues (~90 GB/s each measured), first-need order:
        #   scalar: txt, spk, xT0 upper half, wq-e01, xT-ss3
        #   sync:   wdq, wdk (gate critical path), wk-e01, wv, e23 halves
        #   gpsimd: xT0 lower half, xT-ss1, xT-ss2
        eh0 = slice(0, 256)
        eh1 = slice(256, 512)
        ch0 = slice(0, 4)
        ch1 = slice(4, 8)
        sl0 = slice(0, 512)
        sl1 = slice(512, 1024)
        sl2 = slice(1024, 1536)
        sl3 = slice(1536, 2048)
        nc.sync.dma_start(out=wdq_sb, in_=wdq)
        nc.sync.dma_start(out=wdk_sb, in_=wdk)
        nc.scalar.dma_start(out=xT_sb[:, ch1, sl0], in_=xT[:, ch1, sl0])
        nc.gpsimd.dma_start(out=xT_sb[:, ch0, sl0], in_=xT[:, ch0, sl0])
        nc.scalar.dma_start(out=wq_sb[:, :, eh0], in_=wq[:, :, eh0])
        nc.sync.dma_start(out=wk_sb[:, :, eh0], in_=wk[:, :, eh0])
        nc.gpsimd.dma_start(out=xT_sb[:, :, sl1], in_=xT[:, :, sl1])
        nc.scalar.dma_start(out=xT_sb[:, :, sl3], in_=xT[:, :, sl3])
        nc.gpsimd.dma_start(out=xT_sb[:, :, sl2], in_=xT[:, :, sl2])
        nc.sync.dma_start(out=wv_sb, in_=wv)
        nc.sync.dma_start(out=wq_sb[:, :, eh1], in_=wq[:, :, eh1])
        nc.sync.dma_start(out=wk_sb[:, :, eh1], in_=wk[:, :, eh1])

        # ---- phase A: pooled text + gates (as v1) ----
        pps = pjp.tile([P, 512], FP32, tag="acc", name="poolps")
        for c in range(DC):
            nc.tensor.matmul(pps[:, c:c + 1],
                             lhsT=txt_sb[:, c * P:(c + 1) * P],
                             rhs=tmask_sb[:, 0:1], start=True, stop=True)
        nc.tensor.matmul(pps[:, DC:DC + 1], lhsT=tmask_sb, rhs=ones_sb,
                         start=True, stop=True)
        # partition-broadcast 1/sum(mask) via a tiny PE outer product — a DMA
        # here would strand behind the big preloads on every queue
        rmsum = smallp.tile([1, 1], BF16, tag="rmsum")
        with nc.allow_low_precision(reason="bf16 mask-count recip"):
            nc.vector.reciprocal(rmsum, pps[0:1, DC:DC + 1])
        ones_row = smallp.tile([1, P], BF16, tag="ones_row")
        nc.vector.memset(ones_row, 1.0)
        rmb_ps = pjp.tile([P, 512], FP32, tag="acc", name="rmbps")
        nc.tensor.matmul(rmb_ps[:, 0:1], lhsT=ones_row, rhs=rmsum,
                         start=True, stop=True)
        rmsumb = smallp.tile([P, 1], FP32, tag="rmsumb")
        nc.vector.tensor_copy(rmsumb, rmb_ps[:, 0:1])
        poolT = smallp.tile([P, DC], BF16, tag="poolT")
        nc.vector.tensor_copy(poolT, pps[:, 0:DC])

        gq_sb = smallp.tile([P, ECH], FP32, tag="gq")
        gk_sb = smallp.tile([P, ECH], FP32, tag="gk")
        for (nm, wd_sb, bd_sb, g_sb) in (
            ("q", wdq_sb, bdq_sb, gq_sb),
            ("k", wdk_sb, bdk_sb, gk_sb),
        ):
            for ec in range(ECH):
                gp = pjp.tile([P, 512], FP32, tag="acc", name=f"g{nm}{ec}")
                for c in range(DC):
                    nc.tensor.matmul(
                        gp[:, 0:1],
                        lhsT=wd_sb[:, c, ec * P:(ec + 1) * P],
                        rhs=poolT[:, c:c + 1],
                        start=(c == 0), stop=(c == DC - 1),
                    )
                nc.scalar.activation(g_sb[:, ec:ec + 1], gp[:, 0:1], AF.Sigmoid,
                                     bias=bd_sb[:, ec:ec + 1], scale=rmsumb)
            nc.vector.tensor_scalar(g_sb, g_sb, 1.0, None, OP.add)
        # fold BOTH gates into Q: scores = (xWk+bk) . [gqk*(xWq+bq)]
        gqk_sb = smallp.tile([P, ECH], FP32, tag="gqk")
        nc.vector.tensor_mul(gqk_sb, gq_sb, gk_sb)
        gqkbq_sb = smallp.tile([P, ECH], FP32, tag="gqkbq")
        nc.vector.tensor_mul(gqkbq_sb, gqk_sb, bq_sb)

        # ---- projection targets + emitters ----
        QT = qkpool.tile([P, HP, S], BF16, tag="QT")
        KTp = qkpool.tile([P, HP, S], BF16, tag="KTp")   # pair halves, no pad
        Vaug = vpool.tile([P, SC, HPC, Dh + 1], BF16, tag="Vaug")
        etAB = etp.tile([P, 2, SC, 1024], BF16, tag="etAB")

        def emit_qk_half(ec, ss, which):
            sl = slice(ss * 512, (ss + 1) * 512)
            if which == "q":
                ps = pjp.tile([P, 512], FP32, tag="acc", name=f"psq{ec}_{ss}")
                for c in range(CC):
                    nc.tensor.matmul(
                        ps,
                        lhsT=wq_sb[:, c, ec * P:(ec + 1) * P],
                        rhs=xT_sb[:, c, sl],
                        start=(c == 0), stop=(c == CC - 1),
                    )
                nc.scalar.activation(
                    QT[:, ec, sl], ps, AF.Identity,
                    bias=gqkbq_sb[:, ec:ec + 1], scale=gqk_sb[:, ec:ec + 1],
                )
            else:
                psk = pjp.tile([P, 512], FP32, tag="acc", name=f"psk{ec}_{ss}")
                for c in range(CC):
                    nc.tensor.matmul(
                        psk,
                        lhsT=wk_sb[:, c, ec * P:(ec + 1) * P],
                        rhs=xT_sb[:, c, sl],
                        start=(c == 0), stop=(c == CC - 1),
                    )
                nc.vector.tensor_scalar(
                    KTp[:, ec, sl], psk, bk_sb[:, ec:ec + 1], None, OP.add)

        def emit_v(t):
            ps = pjp.tile([P, 512], FP32, tag="acc", name=f"psv{t}")
            for c in range(CC):
                nc.tensor.matmul(
                    ps,
                    lhsT=xT_sb[:, c, t * P:(t + 1) * P],
                    rhs=wv_sb[:, c, :],
                    start=(c == 0), stop=(c == CC - 1),
                )
            nc.vector.tensor_copy(
                Vaug[:, t, :, 0:Dh],
                ps.rearrange("p (h d) -> p h d", h=HPC),
            )
            nc.vector.memset(Vaug[:, t, :, Dh:Dh + 1], 1.0)

        # only the first block's score inputs run up front (K before Q:
        # K eviction is gate-free so it can start while gates resolve)
        emit_qk_half(0, 0, "k")
        for ec, ss, which in ((0, 0, "q"), (0, 1, "q")):
            emit_qk_half(ec, ss, which)

        # ---- projection dose map: (block, iter) -> emitters ----
        pdose = defaultdict(list)

        def addp(n, i, fn):
            pdose[(n, min(i, 16))].append(fn)

        # block 0 consumes seq-blocks in DMA arrival order: ss0, ss3, ss1, ss2
        PERM0 = (0, 1, 2, 3, 12, 13, 14, 15, 4, 5, 6, 7, 8, 9, 10, 11)
        addp(0, 3, lambda: emit_qk_half(0, 3, "k"))
        addp(0, 5, lambda: emit_qk_half(0, 2, "q"))
        addp(0, 7, lambda: emit_qk_half(0, 1, "k"))
        addp(0, 8, lambda: emit_qk_half(0, 3, "q"))
        addp(0, 11, lambda: emit_qk_half(0, 2, "k"))
        for i in range(SC):
            addp(0, 6 + (i * 9) // 16, lambda t=PERM0[i]: emit_v(t))
        for g in (1, 2, 3):
            bb = 2 * g - 1
            for k in range(4):
                addp(bb, 2 + 2 * k, lambda g=g, k=k: emit_qk_half(g, k, "k"))
            addp(bb, 12, lambda g=g: emit_qk_half(g, 0, "q"))
            addp(bb, 13 if g == 3 else 14, lambda g=g: emit_qk_half(g, 1, "q"))
            if g < 3:
                addp(2 * g, 9, lambda g=g: emit_qk_half(g, 2, "q"))
                addp(2 * g, 11, lambda g=g: emit_qk_half(g, 3, "q"))
            else:
                addp(5, 14, lambda: emit_qk_half(3, 2, "q"))
                addp(5, 15, lambda: emit_qk_half(3, 3, "q"))

        # ---- ctx dose iteration maps (by emit INDEX into the block's perm) --
        def a_iter(i):
            return max(9 + (i * 7) // 16, i + 1)

        def b_iter(i):
            # MUST be <= position of the overwriting exp: block n's ctx-B
            # unit i (t = perm[i]) is emitted in block n+1 before
            # exp-B(n+1, t) overwrites etAB[:,1,t] (program-order WAR).
            return i // 2

        def perm_of(n):
            return PERM0 if n == 0 else tuple(range(SC))

        # ---- attention ----
        ctx_ab = {}      # (n, 'A'/'B') -> [j0_tile, j1_tile]
        cs_store = {}    # (n, 'A'/'B') -> (head, sp, cs_tile)

        def ctx_unit(n, ab, i, pool):
            hp, sp = n // 2, n % 2
            h = 2 * hp + (0 if ab == "A" else 1)
            t = perm_of(n)[i]
            key = (n, ab)
            if key not in ctx_ab:
                ctx_ab[key] = [
                    pool.tile([P, 512], FP32, tag="acc",
                              name=f"ctx{n}{ab}{j}")
                    for j in range(2)]
            for j in range(2):
                nc.tensor.matmul(
                    ctx_ab[key][j][0:Dh + 1, :],
                    lhsT=Vaug[:, t, h, :],
                    rhs=etAB[:, h % 2, t, j * 512:(j + 1) * 512],
                    start=(i == 0), stop=(i == SC - 1),
                )

        def ctx_evict(n, ab):
            hp, sp = n // 2, n % 2
            h = 2 * hp + (0 if ab == "A" else 1)
            cs = rbp.tile([Dh + 1, 1024], BF16, tag="cs")
            for j in range(2):
                nc.vector.tensor_copy(
                    cs[:, j * 512:(j + 1) * 512], ctx_ab[(n, ab)][j][0:Dh + 1, :])
            cs_store[(n, ab)] = (h, sp, cs)

        def phase_e(units):
            """units: list of (h, sp, cs, rowbase in {0,64}). 1 or 2 units."""
            rpack = rcp.tile([P, 16], BF16, tag="rpack")
            for (h, sp, cs, rb) in units:
                nc.sync.dma_start(out=rpack[:, rb // 8:rb // 8 + 8],
                                  in_=cs[Dh:Dh + 1, :])
            rpb = rcp.tile([P, 16], BF16, tag="rpb")
            with nc.allow_low_precision(reason="bf16 softmax denom recip"):
                nc.vector.reciprocal(rpb, rpack)
            rrow = rcp.tile([P, 1024], BF16, tag="rrow")
            for (h, sp, cs, rb) in units:
                nc.sync.dma_start(out=rrow[rb:rb + 1, :],
                                  in_=rpb[:, rb // 8:rb // 8 + 8])
            for j in range(2):
                rcb = ctxp.tile([P, 512], FP32, tag="acc",
                                name=f"rcb{units[0][0]}_{units[0][1]}_{j}")
                for (h, sp, cs, rb) in units:
                    nc.tensor.matmul(
                        rcb[rb:rb + Dh, :], lhsT=ones2[rb:rb + 1, :],
                        rhs=rrow[rb:rb + 1, j * 512:(j + 1) * 512],
                        start=True, stop=True,
                        tile_position=(rb, rb),
                    )
                for (h, sp, cs, rb) in units:
                    hp2, hi = h // 2, h % 2
                    ot = outp.tile([Dh, 512], FP32, tag="outsb")
                    nc.vector.tensor_mul(
                        ot, cs[0:Dh, j * 512:(j + 1) * 512], rcb[rb:rb + Dh, :])
                    nc.vector.tensor_scalar(
                        ot, ot,
                        bvT_sb[hi * Dh:(hi + 1) * Dh, hp2:hp2 + 1], None,
                        OP.add,
                    )
                    nc.gpsimd.dma_start(
                        out=out[h, :, sp * 1024 + j * 512:
                                sp * 1024 + (j + 1) * 512], in_=ot)

        for n in range(8):
            hp, sp = n // 2, n % 2
            perm = perm_of(n)
            for pos in range(SC):
                t = perm[pos]
                with tc.high_priority(offset=-100):
                    if pos == 0 and n >= 1:
                        ctx_evict(n - 1, "A")
                    if n >= 1:
                        for ib in range(SC):
                            if b_iter(ib) == pos:
                                ctx_unit(n - 1, "B", ib, ctxp)
                    if n == 7 and 1 <= pos <= 15:
                        ctx_unit(7, "B", pos - 1, pjp)
                    if pos == 9 and n >= 1:
                        # normalize the whole previous block: {A(n-1), B(n-1)}
                        ctx_evict(n - 1, "B")
                        hA, spA, csA = cs_store.pop((n - 1, "A"))
                        hB, spB, csB = cs_store.pop((n - 1, "B"))
                        phase_e([(hA, spA, csA, 0), (hB, spB, csB, 64)])
                    for fn in pdose.get((n, pos), ()):
                        fn()
                    for ia in range(SC):
                        if a_iter(ia) == pos:
                            ctx_unit(n, "A", ia, ctxp)
                # scores: packed pairs (parity = PE row quadrant), j-split so
                # exp(t,j) pipelines against scores of the next j-half
                for j in range(2):
                    sps = scp.tile([P, 2, 512], FP32, tag="sc")
                    for par in range(2):
                        pp = slice(par * 64, (par + 1) * 64)
                        s0 = sp * 1024 + j * 512
                        # explicit quadrant: pool-tile reg offsets would
                        # otherwise force tile_position (0,0) = no packing
                        nc.tensor.matmul(
                            sps[:, par, :],
                            lhsT=KTp[pp, hp, t * P:(t + 1) * P],
                            rhs=QT[pp, hp, s0:s0 + 512],
                            start=True, stop=True,
                            tile_position=(par * 64, 0),
                        )
                    ets = slice(j * 512, (j + 1) * 512)
                    for par in range(2):
                        if par == 1 and t not in ACT_B_T:
                            nc.vector.tensor_scalar(
                                etAB[:, par, t, ets].bitcast(I16),
                                sps[:, par, :],
                                SCHR_S1, s2_sb[:, t:t + 1], OP.mult, OP.add)
                        else:
                            nc.scalar.activation(
                                etAB[:, par, t, ets], sps[:, par, :], AF.Exp,
                                bias=amask_sb[:, t:t + 1], scale=0.125)
            # end of t loop: position-16 doses
            with tc.high_priority(offset=-100):
                for ia in range(SC):
                    if a_iter(ia) == 16:
                        ctx_unit(n, "A", ia, ctxp)
                for fn in pdose.get((n, 16), ()):
                    fn()
                if n == 7:
                    ctx_unit(7, "B", 15, pjp)

        # tail: last evictions + one paired phase-E
        ctx_evict(7, "A")
        ctx_evict(7, "B")
        hA, spA, csA = cs_store.pop((7, "A"))
        hB, spB, csB = cs_store.pop((7, "B"))
        phase_e([(hA, spA, csA, 0), (hB, spB, csB, 64)])


def _build():
    key = "nc"
    if key in _CACHE:
        return _CACHE[key]
    nc = bacc.Bacc("TRN2", target_bir_lowering=False, debug=False,
                   enable_asserts=False)
    aps = {}

    def din(name, shape, dt):
        aps[name] = nc.dram_tensor(name, shape, dt, kind="ExternalInput").ap()

    din("xT", [DV, S], BF16)
    din("wq", [DV, E], BF16)
    din("wk", [DV, E], BF16)
    din("wv", [DV, E], BF16)
    din("wdq", [DT, E], BF16)
    din("wdk", [DT, E], BF16)
    din("txt", [T, DT], BF16)
    # amask[0:16] | bq | bk | bdq | bdk | bv | schraudolph-s2[36:52] | tmask
    din("smallpack", [P, 2 * SC + 5 * ECH + 1], FP32)
    aps["out"] = nc.dram_tensor("out", [HPC, Dh, S], FP32,
                                kind="ExternalOutput").ap()

    with tile.TileContext(nc) as tc:
        _emit(tc, aps)
    nc.compile()
    _CACHE[key] = nc
    return nc


def kernel(**inputs):
    global last_results
    hs = np.asarray(inputs["hidden_states"], dtype=np.float32)
    amask = np.asarray(inputs["attention_mask"], dtype=np.float32)
    txt = np.asarray(inputs["txt_embedding"], dtype=np.float32)
    tmask = np.asarray(inputs["txt_attention_mask"], dtype=np.float32)
    Wq = np.asarray(inputs["Wq"], dtype=np.float32)
    Wk = np.asarray(inputs["Wk"], dtype=np.float32)
    Wv = np.asarray(inputs["Wv"], dtype=np.float32)
    Wdq = np.asarray(inputs["Wdq"], dtype=np.float32)
    Wdk = np.asarray(inputs["Wdk"], dtype=np.float32)
    bq = np.asarray(inputs["bq"], dtype=np.float32)
    bk = np.asarray(inputs["bk"], dtype=np.float32)
    bv = np.asarray(inputs["bv"], dtype=np.float32)
    bdq = np.asarray(inputs["bdq"], dtype=np.float32)
    bdk = np.asarray(inputs["bdk"], dtype=np.float32)

    nc = _build()

    in_maps = []
    for c in range(NCORES):
        b, g = c // 2, c % 2
        cols = slice(g * E, (g + 1) * E)
        amask_t = amask[b, 0, 0].reshape(SC, P).T      # [128, 16]
        in_maps.append({
            "xT": np.ascontiguousarray(hs[b].T).astype(BF16_NP),
            "wq": Wq[:, cols].astype(BF16_NP),
            "wk": Wk[:, cols].astype(BF16_NP),
            "wv": Wv[:, cols].astype(BF16_NP),
            "wdq": Wdq[:, cols].astype(BF16_NP),
            "wdk": Wdk[:, cols].astype(BF16_NP),
            "txt": txt[b].astype(BF16_NP),
            "smallpack": np.ascontiguousarray(np.concatenate([
                amask_t,
                bq[cols].reshape(ECH, P).T,
                bk[cols].reshape(ECH, P).T,
                bdq[cols].reshape(ECH, P).T,
                bdk[cols].reshape(ECH, P).T,
                bv[cols].reshape(ECH, P).T,
                amask_t * SCHR_AM + SCHR_B0,
                np.pad(tmask[b, :, 0], (0, P - T)).reshape(P, 1),
            ], axis=1).astype(np.float32)),
        })

    tr = int(os.environ.get("BASS_KERNEL_TRACE", "0"))
    if tr == 2:
        run_bass_kernel_spmd(nc, in_maps, list(range(NCORES)), trace=False)
    res = run_bass_kernel_spmd(nc, in_maps, list(range(NCORES)), trace=bool(tr))
    last_results = res

    outp = np.empty((B, S, DV), dtype=np.float32)
    for c in range(NCORES):
        b, g = c // 2, c % 2
        co = res.results[c]["out"].transpose(2, 0, 1).reshape(S, E)
        outp[b, :, g * E:(g + 1) * E] = co
    return outp
